# revision 29
# baseline (speedup 1.0000x reference)
"""Sparse windowed attention kernel for Trainium2 (8 NeuronCores, batch data-parallel).

Computes, per batch b:
    S = (q @ k.T) / 16, masked to the window [prev[b], prev[b]+100)
    p = softmax(S)  over the window
    result     = concat(p @ v, q)           [T, 512]
    alignments = p.T scattered into [N, T]  (zeros outside the window)
    max_att    = argmax(p)                  [T]

Only the 100-wide key window is ever touched: k/v rows are loaded with a
dynamic (register) DMA offset, and p.T is written back to the alignments
rows [prev, prev+100) with a dynamic-offset DMA.  The remaining alignment
rows stay zero via the runtime's pre-zeroed output buffers.
"""

import os
import sys

for _p in ("/opt/trn_rl_repo", "/root/.axon_site/_ro/trn_rl_repo"):
    if os.path.isdir(_p) and _p not in sys.path:
        sys.path.insert(0, _p)
        break

import numpy as np

import concourse.bacc as bacc
import concourse.tile as tile
from concourse import mybir
from concourse.bass import ds, ts
from concourse.bass_utils import run_bass_kernel_spmd
from concourse.masks import make_identity

F32 = mybir.dt.float32
I32 = mybir.dt.int32
U32 = mybir.dt.uint32

NCORES = 8
B = 32
BL = B // NCORES  # batches per core
T = 1024
N = 1024
D = 256
WIN = 100
SCALE = 1.0 / 16.0  # 1/sqrt(D)
NT = T // 128  # 8 t-tiles of 128 rows
NP = NT // 2  # 4 tile-pairs


def _body(tc, q, k, v, prev, res, align, mxa, opt=None):
    opt = opt or {}
    nc = tc.nc
    Exp = mybir.ActivationFunctionType.Exp
    q_v = q.rearrange("b (g p) d -> b p g d", p=128)  # [BL, 128, 8, 256]
    res_v = res.rearrange("b (g p) d -> b p g d", p=128)  # [BL, 128, 8, 512]
    mxa_v = mxa.rearrange("b (g p) -> b g p", p=128)  # [BL, 8, 128]

    with (
        tc.tile_pool(name="constp", bufs=1) as constp,
        tc.tile_pool(name="iop", bufs=opt.get("iop", 3)) as iop,
        tc.tile_pool(name="qtp", bufs=opt.get("qtp", 3)) as qtp,
        tc.tile_pool(name="smp", bufs=opt.get("smp", 3)) as smp,
        tc.tile_pool(name="kvp", bufs=opt.get("kvp", 2)) as kvp,
        tc.tile_pool(name="ptp", bufs=opt.get("ptp", 2)) as ptp,
        tc.tile_pool(name="ps_qt", bufs=opt.get("ps_qt", 2), space="PSUM") as ps_qt,
        tc.tile_pool(name="ps_s", bufs=opt.get("ps_s", 2), space="PSUM") as ps_s,
        tc.tile_pool(name="ps_t", bufs=opt.get("ps_t", 2), space="PSUM") as ps_t,
        tc.tile_pool(name="ps_av", bufs=opt.get("ps_av", 2), space="PSUM") as ps_av,
    ):
        ident = constp.tile([128, 128], F32)
        make_identity(nc, ident)

        prev_sb = constp.tile([1, BL], I32)
        nc.sync.dma_start(prev_sb, prev)
        prev_f1 = constp.tile([1, BL], F32)
        nc.vector.tensor_copy(prev_f1, prev_sb)
        prev_f8 = constp.tile([8, BL], F32)
        nc.gpsimd.partition_broadcast(prev_f8, prev_f1)

        if opt.get("warm_exp", False):
            # touch Exp once so the ACT table set loads during DMA warmup
            warm = constp.tile([1, 1], F32)
            nc.scalar.activation(warm, ident[0:1, 0:1], Exp)

        gs_top = opt.get("gs", 2)
        o_grps = {}
        if opt.get("hoist_first", False):
            o_grp0 = iop.tile([128, gs_top, 2 * D], F32, name="o_grp_first", tag="o_grp")
            nc.sync.dma_start(o_grp0[:, :, D : 2 * D], q_v[0, :, 0:gs_top, :])
            o_grps[(0, 0)] = o_grp0
        if opt.get("hoist_q", False):
            for b in range(BL):
                for gi in range(NT // gs_top):
                    o_grp = iop.tile([128, gs_top, 2 * D], F32, name=f"o_grp_{b}_{gi}", tag="o_grp")
                    nc.sync.dma_start(
                        o_grp[:, :, D : 2 * D],
                        q_v[b, :, gs_top * gi : gs_top * gi + gs_top, :],
                    )
                    o_grps[(b, gi * gs_top)] = o_grp

        from concourse.ordered_set import OrderedSet

        rv_engines = OrderedSet(
            [mybir.EngineType.SP, mybir.EngineType.Pool]
            if (opt.get("store_gpsimd", True) or opt.get("tail_gpsimd", False))
            else [mybir.EngineType.SP]
        )

        gs_opt = opt.get("gs", 2)  # t-tiles per group
        store_eng = nc.gpsimd if opt.get("store_gpsimd", True) else nc.sync
        tail_eng = nc.gpsimd if opt.get("tail_gpsimd", False) else store_eng

        def batch_setup(b):
            rv = nc.values_load(
                prev_sb[0:1, b : b + 1],
                min_val=0,
                max_val=N - WIN,
                skip_runtime_bounds_check=True,
                engines=rv_engines,
            )

            k_sb = kvp.tile([128, D], F32, name="k_sb", tag="k_sb")
            v_sb = kvp.tile([128, D], F32, name="v_sb", tag="v_sb")
            nc.sync.dma_start(k_sb[0:WIN, :], k[b][ds(rv, WIN), :])
            nc.sync.dma_start(v_sb[0:WIN, :], v[b][ds(rv, WIN), :])

            # k window transposed: kT[dchunk][d, w]
            kT_ps = ps_t.tile([128, 2, WIN], F32, tag="ps_t", name="kT_ps")
            nc.tensor.transpose(kT_ps[:, 0, :], k_sb[0:WIN, 0:128], ident[0:WIN, 0:WIN])
            nc.tensor.transpose(kT_ps[:, 1, :], k_sb[0:WIN, 128:256], ident[0:WIN, 0:WIN])
            kT_sb = kvp.tile([128, 2, WIN], F32, name="kT_sb", tag="kT_sb")
            nc.vector.tensor_copy(kT_sb, kT_ps)

            pT_full = ptp.tile([128, T], F32, name="pT_full")
            idx_all = smp.tile([128, NT, 8], U32, name="idx_all", tag="idx_all")
            return rv, v_sb, kT_sb, pT_full, idx_all

        def process_group(b, gi_, gi, gs, tile0, st):
            rv, v_sb, kT_sb, pT_full, idx_all = st
            if True:
                if (b, tile0) in o_grps and gs == gs_top:
                    o_grp = o_grps[(b, tile0)]
                else:
                    o_grp = iop.tile([128, gs, 2 * D], F32, tag="o_grp")
                    if b == 0 and gi_ == 0 and opt.get("split_first_qload", False) and gs >= 2:
                        h2 = gs // 2
                        nc.sync.dma_start(
                            o_grp[:, 0:h2, D : 2 * D], q_v[b, :, tile0 : tile0 + h2, :]
                        )
                        nc.sync.dma_start(
                            o_grp[:, h2:gs, D : 2 * D],
                            q_v[b, :, tile0 + h2 : tile0 + gs, :],
                        )
                    else:
                        nc.sync.dma_start(
                            o_grp[:, :, D : 2 * D], q_v[b, :, tile0 : tile0 + gs, :]
                        )

                # transpose the group's q tiles (2 d-chunks each) -> qT [d, t]
                qt_sb = qtp.tile([128, 2 * gs, 128], F32)
                for h in range(gs // 2):  # 2 t-tiles per psum bank
                    qt_ps = ps_qt.tile([128, 4, 128], F32)
                    for g in range(2):
                        for dc in range(2):
                            nc.tensor.transpose(
                                qt_ps[:, 2 * g + dc, :],
                                o_grp[:, 2 * h + g, D + dc * 128 : D + (dc + 1) * 128],
                                ident,
                            )
                    nc.any.tensor_copy(qt_sb[:, 4 * h : 4 * h + 4, :], qt_ps)

                if opt.get("per_tile", False):
                    # fully per-tile softmax pipeline (finest dependencies)
                    e_list = []
                    for g in range(gs):
                        s_ps = ps_s.tile([128, WIN], F32, tag="s_ps")
                        nc.tensor.matmul(
                            s_ps, qt_sb[:, 2 * g, :], kT_sb[:, 0, :],
                            start=True, stop=False,
                        )
                        nc.tensor.matmul(
                            s_ps, qt_sb[:, 2 * g + 1, :], kT_sb[:, 1, :],
                            start=False, stop=True,
                        )
                        e_sb = smp.tile([128, WIN], F32, tag="e_sb")
                        ssum = smp.tile([128, 1], F32, tag="ssum")
                        nc.scalar.activation(
                            e_sb, s_ps, Exp, scale=SCALE, accum_out=ssum
                        )
                        r_t = smp.tile([128, 1], F32, tag="r_t")
                        nc.vector.reciprocal(r_t, ssum)
                        p_sb = smp.tile([128, WIN], F32, tag="p_sb")
                        nc.vector.tensor_scalar_mul(p_sb, e_sb, r_t)
                        pT_ps = ps_t.tile([128, 128], F32, tag="ps_t")
                        nc.tensor.transpose(pT_ps[0:WIN, :], p_sb, ident)
                        nc.vector.tensor_copy(
                            pT_full[0:WIN, ts(tile0 + g, 128)], pT_ps[0:WIN, :]
                        )
                        e_list.append(e_sb)
                    e_view = lambda g: e_list[g]
                else:
                    # scores for the whole group into one psum bank
                    s_ps = ps_s.tile([128, gs, WIN], F32)
                    for g in range(gs):
                        nc.tensor.matmul(
                            s_ps[:, g, :],
                            qt_sb[:, 2 * g, :],
                            kT_sb[:, 0, :],
                            start=True,
                            stop=False,
                        )
                        nc.tensor.matmul(
                            s_ps[:, g, :],
                            qt_sb[:, 2 * g + 1, :],
                            kT_sb[:, 1, :],
                            start=False,
                            stop=True,
                        )

                    e4 = smp.tile([128, gs, WIN], F32)
                    ssum4 = smp.tile([128, gs], F32)
                    for g in range(gs):
                        nc.scalar.activation(
                            e4[:, g, :],
                            s_ps[:, g, :],
                            Exp,
                            scale=SCALE,
                            accum_out=ssum4[:, g : g + 1],
                        )
                    r4 = smp.tile([128, gs], F32)
                    p4 = smp.tile([128, gs, WIN], F32)
                    if opt.get("fine_sm", False):
                        for g in range(gs):
                            nc.vector.reciprocal(r4[:, g : g + 1], ssum4[:, g : g + 1])
                            nc.vector.tensor_scalar_mul(
                                p4[:, g, :], e4[:, g, :], r4[:, g : g + 1]
                            )
                    else:
                        nc.vector.reciprocal(r4, ssum4)
                        nc.vector.tensor_tensor(
                            p4, e4, r4.to_broadcast((128, gs, WIN)), mybir.AluOpType.mult
                        )

                    # p.T for the group: alignments rows + AV stationary operand
                    pt_ps = ps_t.tile([128, gs, 128], F32, tag="ps_t")
                    for g in range(gs):
                        nc.tensor.transpose(pt_ps[0:WIN, g, :], p4[:, g, :], ident)
                    if opt.get("fine_pt", False):
                        for g in range(gs):
                            nc.vector.tensor_copy(
                                pT_full[0:WIN, ts(tile0 + g, 128)], pt_ps[0:WIN, g, :]
                            )
                    else:
                        nc.vector.tensor_copy(
                            pT_full[0:WIN, gs * 128 * gi : gs * 128 * (gi + 1)],
                            pt_ps[0:WIN, :, :],
                        )
                    e_view = lambda g: e4[:, g, :]

                def do_argmax():
                    # top-1 index per row (first occurrence on ties)
                    for g in range(gs):
                        tidx = tile0 + g
                        vmax8 = smp.tile([128, 8], F32, name="vmax8", tag="vmax8")
                        nc.vector.max_with_indices(
                            vmax8, idx_all[:, tidx, :], e_view(g)
                        )

                if opt.get("argmax_first", False):
                    do_argmax()

                for pr in range(gs // 2):  # AV matmuls per tile-pair
                    av_ps = ps_av.tile([128, 2, D], F32)
                    for g in range(2):
                        tidx = tile0 + 2 * pr + g
                        nc.tensor.matmul(
                            av_ps[:, g, :],
                            pT_full[0:WIN, ts(tidx, 128)],
                            v_sb[0:WIN, :],
                            start=True,
                            stop=True,
                        )
                    nc.any.tensor_copy(
                        o_grp[:, 2 * pr : 2 * pr + 2, 0:D], av_ps
                    )

                if not opt.get("argmax_first", False):
                    do_argmax()

                if opt.get("split_store", False):
                    for pr in range(gs // 2):
                        store_eng.dma_start(
                            res_v[b, :, tile0 + 2 * pr : tile0 + 2 * pr + 2, :],
                            o_grp[:, 2 * pr : 2 * pr + 2, :],
                        )
                else:
                    store_eng.dma_start(res_v[b, :, tile0 : tile0 + gs, :], o_grp)

        def batch_finalize(b, st):
            rv, v_sb, kT_sb, pT_full, idx_all = st
            # alignments window rows: contiguous dynamic-offset store
            if opt.get("split_align", False):
                for hh in range(2):
                    tail_eng.dma_start(
                        align[b][ds(rv, WIN), hh * 512 : (hh + 1) * 512],
                        pT_full[0:WIN, hh * 512 : (hh + 1) * 512],
                    )
            else:
                tail_eng.dma_start(align[b][ds(rv, WIN), :], pT_full[0:WIN, :])

            # finalize argmax: local idx -> [8, 128] layout, + prev, -> int32
            loc_f = smp.tile([128, NT], F32, name="loc_f", tag="loc_f")
            nc.vector.tensor_copy(loc_f, idx_all[:, :, 0:1].rearrange("p t 1 -> p t"))
            idxT_ps = ps_t.tile([8, 128], F32, tag="ps_t", name="idxT_ps")
            nc.tensor.transpose(idxT_ps, loc_f, ident)
            gidx = smp.tile([8, 128], I32, name="gidx", tag="gidx")
            nc.vector.tensor_scalar(
                gidx, idxT_ps, prev_f8[0:8, b : b + 1], None, mybir.AluOpType.add
            )
            tail_eng.dma_start(mxa_v[b], gidx)

        def group_sizes_for(b):
            if b == 0 and opt.get("first_small", False):
                return [2, 2] + [gs_opt] * ((NT - 4) // gs_opt)
            return [gs_opt] * (NT // gs_opt)

        if opt.get("interleave", False):
            states = {b: batch_setup(b) for b in range(BL)}
            n_groups = NT // gs_opt
            for gi_ in range(n_groups):
                for b in range(BL):
                    process_group(b, gi_, gi_, gs_opt, gi_ * gs_opt, states[b])
                    if gi_ == n_groups - 1:
                        batch_finalize(b, states[b])
        else:
            for b in range(BL):
                st = batch_setup(b)
                tile_base = 0
                for gi_, gs in enumerate(group_sizes_for(b)):
                    process_group(b, gi_, tile_base // gs, gs, tile_base, st)
                    tile_base += gs
                batch_finalize(b, st)


DEFAULT_OPT = {
    "gs": 4,
    "per_tile": True,
    "store_gpsimd": False,
    "tail_gpsimd": True,
    "split_store": True,
    "split_align": True,
    "ps_s": 3,
    "ps_av": 1,
    "iop": 6,
    "qtp": 6,
    "smp": 12,
    "kvp": 6,
    "ptp": 3,
}


def build_nc(opt=None):
    nc = bacc.Bacc(
        "TRN2",
        target_bir_lowering=False,
        debug=False,
        enable_asserts=False,
        num_devices=NCORES,
    )
    q = nc.dram_tensor("query", [BL, T, D], F32, kind="ExternalInput").ap()
    k = nc.dram_tensor("key", [BL, N, D], F32, kind="ExternalInput").ap()
    v = nc.dram_tensor("value", [BL, N, D], F32, kind="ExternalInput").ap()
    prev = nc.dram_tensor("prev", [1, BL], I32, kind="ExternalInput").ap()
    res = nc.dram_tensor("result", [BL, T, 2 * D], F32, kind="ExternalOutput").ap()
    align = nc.dram_tensor("alignments", [BL, N, T], F32, kind="ExternalOutput").ap()
    mxa = nc.dram_tensor("max_attentions", [BL, T], I32, kind="ExternalOutput").ap()

    with tile.TileContext(nc) as tc:
        _body(tc, q, k, v, prev, res, align, mxa, opt=DEFAULT_OPT if opt is None else opt)
    nc.compile()
    return nc


_NC = None


def _get_nc():
    global _NC
    if _NC is None:
        _NC = build_nc()
    return _NC


def make_in_maps(query, key, value, prev_max_attention):
    query = np.ascontiguousarray(np.asarray(query), dtype=np.float32)
    key = np.ascontiguousarray(np.asarray(key), dtype=np.float32)
    value = np.ascontiguousarray(np.asarray(value), dtype=np.float32)
    prev = np.asarray(prev_max_attention).astype(np.int32).reshape(NCORES, 1, BL)
    in_maps = []
    for c in range(NCORES):
        sl = slice(c * BL, (c + 1) * BL)
        in_maps.append(
            {
                "query": query[sl],
                "key": key[sl],
                "value": value[sl],
                "prev": np.ascontiguousarray(prev[c]),
            }
        )
    return in_maps


def gather_outputs(results):
    result = np.concatenate([r["result"] for r in results], axis=0)
    alignments = np.concatenate([r["alignments"] for r in results], axis=0)
    max_att = np.concatenate([r["max_attentions"] for r in results], axis=0)
    return result, alignments, max_att


def kernel(query, key, value, prev_max_attention):
    nc = _get_nc()
    in_maps = make_in_maps(query, key, value, prev_max_attention)
    out = run_bass_kernel_spmd(nc, in_maps, core_ids=list(range(NCORES)))
    return gather_outputs(out.results)

